# revision 31
# baseline (speedup 1.0000x reference)
"""Bahdanau-attention scores kernel for Trainium2, 8-core data-parallel.

Computes softmax_s( v . tanh(W_h @ h[b] + W_e @ enc[s,b] + bias) ) for
B=32, S=2048, Dd=512, De2=1024, sharded 4 batches per NeuronCore.

Two-precision scheme (single launch):
  Pass 1 (fp8):  E^T = W_e8 @ enc8 on the PE in e4m3 DoubleRow mode
                 (K=256 per pass, 2x fp16 rate). tanh on ACT with
                 scale=1/128 dequant + per-partition h-projection bias,
                 v-weighted sum via DVE tree, scores via ones-matmul.
  Select:        per batch row, scores land as [16,128] (DRAM-roundtrip
                 relayout); top-8 of each 128-chunk via max_with_indices
                 = 128 candidates/row. fp8 score error (~0.2) only
                 matters for positions near the row max; top-8/chunk
                 covers everything with softmax weight > ~e^-6.
  Refine (fp16): gather the 128 selected enc rows (indirect DMA), PE
                 transpose, recompute scores in fp16, exp, and merge
                 back into the fp8 exp row via 8 predicated copies.
  Finalize:      row sum of merged exp -> reciprocal -> scale -> DMA.

The h-projection (hidden @ W_h^T + bias) is precomputed on host in
exact f32 and shipped as a per-partition bias table.
"""

import numpy as np

B = 32
S = 2048
DD = 512
DE2 = 1024
NCORES = 8
BL = B // NCORES  # 4 batches per core
R = BL * S  # 8192 rows per core
NK = DE2 // 128  # 8 k-chunks
NO = DD // 128  # 4 o-chunks
NB2 = R // 1024  # 8 DMA blocks of 1024 rows
EXP_OFF = -26.0  # softmax shift; scores observed in [-32, 27]
W8SCALE = 128.0  # fp8 weight pre-scale (keeps W_e out of e4m3 subnormals)
NWARM = 88

_CACHE = {}


def _build_bass():
    import concourse.bacc as bacc
    import concourse.mybir as mybir
    import concourse.tile as tile
    import concourse.bass as bass
    import concourse.bass_isa as bass_isa
    from concourse._compat import get_trn_type

    f32 = mybir.dt.float32
    f16 = mybir.dt.float16
    f8 = mybir.dt.float8e4
    i32 = mybir.dt.int32
    u32 = mybir.dt.uint32
    AF = mybir.ActivationFunctionType
    DR = mybir.MatmulPerfMode.DoubleRow

    nc = bacc.Bacc(get_trn_type() or "TRN2", target_bir_lowering=False, debug=False)

    encB8 = nc.dram_tensor("encB8", [128, NB2 * NK * 1024], f8, kind="ExternalInput")
    w8 = nc.dram_tensor("w8", [128, NO * NK * 128], f8, kind="ExternalInput")
    w16 = nc.dram_tensor("w16", [128, NO * NK * 128], f16, kind="ExternalInput")
    hb_in = nc.dram_tensor("hb_in", [128, NO * BL], f32, kind="ExternalInput")
    v_pb = nc.dram_tensor("v_pb", [128, NO], f32, kind="ExternalInput")
    encP16 = nc.dram_tensor("encP16", [R, DE2], f16, kind="ExternalInput")
    ident16 = nc.dram_tensor("ident16", [128, 128], f16, kind="ExternalInput")
    iotac_in = nc.dram_tensor("iotac_in", [16, 128], f32, kind="ExternalInput")
    posb_in = nc.dram_tensor("posb_in", [16, BL], f32, kind="ExternalInput")
    probs = nc.dram_tensor("probs", [BL, S], f32, kind="ExternalOutput")
    scr_dram = nc.dram_tensor("scr_dram", [BL, S], f32, kind="Internal")
    scl_dram = nc.dram_tensor("scl_dram", [BL, 128], f32, kind="Internal")
    idx_dram = nc.dram_tensor("idx_dram", [BL, 128], i32, kind="Internal")

    with tile.TileContext(nc) as tc:
        with (
            tc.tile_pool(name="const", bufs=1) as const,
            tc.tile_pool(name="encp", bufs=3) as encp,
            tc.tile_pool(name="etp", bufs=4) as etp,
            tc.tile_pool(name="prp", bufs=5) as prp,
            tc.tile_pool(name="refp", bufs=2) as refp,
            tc.tile_pool(name="pep", bufs=3, space="PSUM") as pep,
            tc.tile_pool(name="pmisc", bufs=1, space="PSUM") as pmisc,
            tc.tile_pool(name="pref", bufs=1, space="PSUM") as pref,
        ):
            # ---- PE warm-up: dummy matmuls while DMAs stream in ----
            warm_sb = const.tile([128, 128], f16, name="warm_sb")
            nc.any.memset(warm_sb[:], 0.0)
            wu_ps = pmisc.tile([128, 128], f32, name="wu_ps", tag="mi")
            for i in range(NWARM):
                nc.tensor.matmul(
                    wu_ps[:], warm_sb[:], warm_sb[:], start=True, stop=True
                )

            # ---- constants / weights (scalar queue) ----
            encB_v = encB8[:].rearrange("p (t k r) -> p t k r", t=NB2, k=NK)
            w8_sb = const.tile([128, NO, NK, 128], f8, name="w8_sb")
            w8_v = w8[:].rearrange("p (j k oo) -> p j k oo", j=NO, k=NK)
            for j in range(NO):
                nc.scalar.dma_start(w8_sb[:, j], w8_v[:, j])
            hb_sb = const.tile([128, NO, BL], f32, name="hb_sb")
            nc.scalar.dma_start(hb_sb[:], hb_in[:].rearrange("p (j b) -> p j b", j=NO))
            v_sb = const.tile([128, NO], f32, name="v_sb")
            nc.scalar.dma_start(v_sb[:], v_pb[:])
            w16_sb = const.tile([128, NO, NK, 128], f16, name="w16_sb")
            nc.scalar.dma_start(
                w16_sb[:], w16[:].rearrange("p (j k oo) -> p j k oo", j=NO, k=NK)
            )
            id_sb = const.tile([128, 128], f16, name="id_sb")
            nc.scalar.dma_start(id_sb[:], ident16[:])
            iotac = const.tile([16, 128], f32, name="iotac")
            nc.scalar.dma_start(iotac[:], iotac_in[:])
            posb = const.tile([16, BL], f32, name="posb")
            nc.scalar.dma_start(posb[:], posb_in[:])

            ones_v = const.tile([128, 1], f16, name="ones_v")
            nc.any.memset(ones_v[:], 1.0)
            ones16 = const.tile([16, 1], f32, name="ones16")
            nc.any.memset(ones16[:], 1.0)
            onesb = const.tile([1, 16], f32, name="onesb")
            nc.any.memset(onesb[:], 1.0)
            expoff16 = const.tile([16, 1], f32, name="expoff16")
            nc.any.memset(expoff16[:], EXP_OFF)
            scrow = [
                const.tile([1, S], f32, name=f"scrow{b}") for b in range(2)
            ]  # double-buffered per-row score rows

            def emit_score(st):
                p0, b0_, u0_ = st
                for h in range(2):
                    t0_ = 2 * u0_ + h
                    sc = pmisc.tile([1, 512], f32, name="sc", tag="mi")
                    nc.tensor.matmul(
                        sc[:], ones_v[:], p0[:, h, :], start=True, stop=True
                    )
                    nc.scalar.copy(
                        scrow[b0_ % 2][0:1, 512 * t0_ : 512 * (t0_ + 1)], sc[:]
                    )

            def emit_refine_a1(b):
                row = scrow[b % 2]
                # relayout scores [1,2048] -> [16,128] via DRAM roundtrip
                nc.gpsimd.dma_start(scr_dram[b : b + 1], row[:])
                sc16 = refp.tile([16, 128], f32, name="sc16", tag="sc16")
                nc.gpsimd.dma_start(
                    sc16[:], scr_dram[b].rearrange("(p t) -> p t", p=16)
                )
                return sc16

            def emit_refine_a2(b, sc16):
                expo16 = refp.tile([16, 128], f32, name="expo16", tag="ex16")
                nc.scalar.activation(expo16[:], sc16[:], AF.Exp, bias=expoff16[:])
                # top-8 per 128-chunk
                m8 = refp.tile([16, 8], f32, name="m8", tag="m8")
                mi = refp.tile([16, 8], u32, name="mi", tag="mi8")
                nc.vector.max_with_indices(m8[:], mi[:], sc16[:])
                mif = refp.tile([16, 8], f32, name="mif", tag="mif")
                nc.vector.tensor_copy(mif[:], mi[:])
                idxgf = refp.tile([16, 8], f32, name="idxgf", tag="idxgf")
                nc.vector.tensor_scalar(
                    idxgf[:], mif[:], posb[:, b : b + 1], None, mybir.AluOpType.add
                )
                idxg = refp.tile([16, 8], i32, name="idxg", tag="idxg")
                nc.vector.tensor_copy(idxg[:], idxgf[:])
                # indices -> [128,1] via DRAM roundtrip, then gather enc rows
                nc.gpsimd.dma_start(
                    idx_dram[b].rearrange("(p j) -> p j", p=16), idxg[:]
                )
                idx128 = refp.tile([128, 1], i32, name="idx128", tag="i128")
                nc.gpsimd.dma_start(
                    idx128[:], idx_dram[b].rearrange("(p j) -> p j", p=128)
                )
                gath = refp.tile([128, DE2], f16, name="gath", tag="gath")
                nc.gpsimd.indirect_dma_start(
                    out=gath[:],
                    out_offset=None,
                    in_=encP16[:],
                    in_offset=bass.IndirectOffsetOnAxis(ap=idx128[:, :1], axis=0),
                )
                return (gath, expo16, mif)

            def emit_refine_b1(b, st):
                gath, expo16, mif = st
                # PE transpose to [128 feat, 128 pos] per k-chunk
                encsel = refp.tile([128, NK, 128], f16, name="encsel", tag="esel")
                tp = pref.tile([128, NK, 128], f16, name="tp", tag="rf")
                for k in range(NK):
                    nc.tensor.transpose(
                        tp[:, k, :], gath[:, 128 * k : 128 * (k + 1)], id_sb[:]
                    )
                nc.vector.tensor_copy(encsel[:], tp[:])
                # fp16 recompute of the 128 selected scores
                ret = []
                rpe = pref.tile([128, NO, 128], f32, name="rpe", tag="rf")
                for j in range(NO):
                    for k in range(NK):
                        nc.tensor.matmul(
                            rpe[:, j, :],
                            w16_sb[:, j, k, :],
                            encsel[:, k, :],
                            start=(k == 0),
                            stop=(k == NK - 1),
                        )
                    rt = refp.tile([128, 128], f16, name="rt", tag=f"rt{j}")
                    nc.scalar.activation(
                        rt[:], rpe[:, j, :], AF.Tanh, bias=hb_sb[:, j, b : b + 1]
                    )
                    ret.append(rt)
                scsel = pref.tile([1, 128], f32, name="scsel", tag="rf")
                for j in range(NO):
                    nc.tensor.matmul(
                        scsel[:],
                        v16_sb[:, j : j + 1],
                        ret[j][:],
                        start=(j == 0),
                        stop=(j == NO - 1),
                    )
                scselS = refp.tile([1, 128], f32, name="scselS", tag="sclS")
                nc.scalar.copy(scselS[:], scsel[:])
                # [1,128] -> [16,8] roundtrip, exp, merge into expo16
                nc.gpsimd.dma_start(scl_dram[b : b + 1], scselS[:])
                scs16 = refp.tile([16, 8], f32, name="scs16", tag="scs16")
                nc.gpsimd.dma_start(
                    scs16[:], scl_dram[b].rearrange("(p j) -> p j", p=16)
                )
                return (scs16, expo16, mif)

            def emit_refine_b2(b, st):
                scs16, expo16, mif = st
                es16 = refp.tile([16, 8], f32, name="es16", tag="es16")
                nc.scalar.activation(es16[:], scs16[:], AF.Exp, bias=expoff16[:])
                for j in range(8):
                    mj = refp.tile([16, 128], mybir.dt.int32, name="mj", tag="mj")
                    nc.vector.tensor_scalar(
                        mj[:], iotac[:], mif[:, j : j + 1], None,
                        mybir.AluOpType.is_equal,
                    )
                    nc.vector.copy_predicated(
                        expo16[:], mj[:], es16[:, j : j + 1].to_broadcast([16, 128])
                    )
                return expo16

            def emit_refine_b3(b, expo16):
                # row sum -> reciprocal -> broadcast -> normalize -> out
                rsum = refp.tile([16, 1], f32, name="rsum", tag="rsum")
                nc.vector.reduce_sum(rsum[:], expo16[:], axis=mybir.AxisListType.X)
                tot = pref.tile([1, 1], f32, name="tot", tag="rf")
                nc.tensor.matmul(tot[:], ones16[:], rsum[:], start=True, stop=True)
                totS = refp.tile([1, 1], f32, name="totS", tag="totS")
                nc.vector.tensor_copy(totS[:], tot[:])
                rec = refp.tile([1, 1], f32, name="rec", tag="rec")
                nc.vector.reciprocal(rec[:], totS[:])
                recb = pref.tile([16, 1], f32, name="recb", tag="rf")
                nc.tensor.matmul(recb[:], onesb[:], rec[:], start=True, stop=True)
                recbS = refp.tile([16, 1], f32, name="recbS", tag="rcbS")
                nc.vector.tensor_copy(recbS[:], recb[:])
                probs16 = refp.tile([16, 128], f32, name="probs16", tag="p16")
                nc.vector.tensor_scalar_mul(probs16[:], expo16[:], recbS[:])
                nc.scalar.dma_start(
                    probs[b].rearrange("(p t) -> p t", p=16), probs16[:]
                )

            v16_sb = const.tile([128, NO], f16, name="v16_sb")
            nc.vector.tensor_copy(v16_sb[:], v_sb[:])

            # ---- main loop: 8 DMA blocks of 1024 rows (= half a batch).
            # All small/dependent ops are emitted AFTER each block's main
            # matmuls so in-order engines never stall ahead of bulk work.
            # refine(r) phases: A1@end(2r+2) A2@end(2r+3) B1@end(2r+4)
            # B2@end(2r+5); later phases spill into the tail.
            pending = []
            ref_st = {}
            for t2 in range(NB2):
                enc_t = encp.tile([128, NK, 1024], f8, name="enc_t", tag="enc")
                nc.sync.dma_start(enc_t[:], encB_v[:, t2])
                b = t2 // 2
                prodacc = None
                for j in range(NO):
                    pe = pep.tile([128, 2, 512], f32, name="pe", tag="pe")
                    for kk in range(NK // 2):
                        for h in range(2):
                            nc.tensor.matmul(
                                pe[:, h, :],
                                w8_sb[:, j, 2 * kk : 2 * kk + 2, :],
                                enc_t[:, 2 * kk : 2 * kk + 2, 512 * h : 512 * (h + 1)],
                                start=(kk == 0),
                                stop=(kk == NK // 2 - 1),
                                perf_mode=DR,
                            )
                    et = etp.tile([128, 2, 512], f16, name="et", tag="et")
                    nc.scalar.activation(
                        et[:],
                        pe[:],
                        AF.Tanh,
                        bias=hb_sb[:, j, b : b + 1],
                        scale=1.0 / W8SCALE,
                    )
                    if j == 0:
                        pa = prp.tile([128, 2, 512], f16, name="pa", tag="pa")
                        nc.vector.tensor_scalar_mul(pa[:], et[:], v_sb[:, 0:1])
                        prodacc = pa
                    else:
                        pj = prp.tile([128, 2, 512], f16, name="pj", tag="pj")
                        nc.vector.tensor_scalar_mul(pj[:], et[:], v_sb[:, j : j + 1])
                        nc.vector.tensor_add(prodacc[:], prodacc[:], pj[:])
                # ---- block-end emissions (deps are a full block old) ----
                for st in pending:
                    emit_score(st)
                pending = []
                pending.append((prodacc, b, t2 % 2))
                r_a1 = (t2 - 2) // 2 if t2 >= 2 and t2 % 2 == 0 else None
                r_a2 = (t2 - 3) // 2 if t2 >= 3 and t2 % 2 == 1 else None
                r_b1 = (t2 - 4) // 2 if t2 >= 4 and t2 % 2 == 0 else None
                r_b2 = (t2 - 5) // 2 if t2 >= 5 and t2 % 2 == 1 else None
                if r_a1 is not None:
                    ref_st[("a", r_a1)] = emit_refine_a1(r_a1)
                if r_b1 is not None:
                    ref_st[("b", r_b1)] = emit_refine_b1(
                        r_b1, ref_st.pop(("a2", r_b1))
                    )
                if r_a2 is not None:
                    ref_st[("a2", r_a2)] = emit_refine_a2(
                        r_a2, ref_st.pop(("a", r_a2))
                    )
                if r_b2 is not None:
                    ref_st[("e", r_b2)] = emit_refine_b2(
                        r_b2, ref_st.pop(("b", r_b2))
                    )
                r_b3 = (t2 - 6) // 2 if t2 >= 6 and t2 % 2 == 0 else None
                if r_b3 is not None:
                    emit_refine_b3(r_b3, ref_st.pop(("e", r_b3)))

            # ---- tail: scores of block 7, then refine rows 2 and 3 ----
            for st in pending:
                emit_score(st)
            emit_refine_b3(1, ref_st.pop(("e", 1)))
            ref_st[("b", 2)] = emit_refine_b1(2, ref_st.pop(("a2", 2)))
            sc16_3 = emit_refine_a1(BL - 1)
            st3 = emit_refine_a2(BL - 1, sc16_3)
            e2 = emit_refine_b2(2, ref_st.pop(("b", 2)))
            e3 = emit_refine_b2(BL - 1, emit_refine_b1(BL - 1, st3))
            emit_refine_b3(2, e2)
            emit_refine_b3(BL - 1, e3)

    nc.compile()
    return nc


def _get_nc():
    if "nc" not in _CACHE:
        _CACHE["nc"] = _build_bass()
    return _CACHE["nc"]


def _tile_rows(mat_t, nchunk):
    # [nchunk*128, F] -> [128, nchunk*F] with out[p, c*F+f] = mat_t[128c+p, f]
    n, F = mat_t.shape
    assert n == nchunk * 128
    return np.ascontiguousarray(
        mat_t.reshape(nchunk, 128, F).transpose(1, 0, 2)
    ).reshape(128, nchunk * F)


def _make_in_maps(hidden, enc, W, b, v):
    import ml_dtypes

    f8 = ml_dtypes.float8_e4m3
    W_h = W[:, :DD]
    W_e = W[:, DD:]
    # w[p, j, k, oo] = W_e[128j+oo, 128k+p]
    w_lay = np.ascontiguousarray(
        W_e.reshape(NO, 128, NK, 128).transpose(3, 0, 2, 1)
    ).reshape(128, NO * NK * 128)
    w8_arr = (w_lay * W8SCALE).astype(f8)
    w16_arr = w_lay.astype(np.float16)
    v_pb = np.ascontiguousarray(v.reshape(NO, 128).T).astype(np.float32)
    ident = np.eye(128, dtype=np.float16)
    iotac = np.broadcast_to(
        np.arange(128, dtype=np.float32), (16, 128)
    ).copy()
    posb = (
        2048.0 * np.arange(BL)[None, :] + 128.0 * np.arange(16)[:, None]
    ).astype(np.float32)
    in_maps = []
    for c in range(NCORES):
        ec = enc[:, BL * c : BL * (c + 1), :]  # [S, BL, DE2]
        encT = np.ascontiguousarray(ec.transpose(2, 1, 0)).reshape(DE2, R)
        encB = np.ascontiguousarray(
            encT.reshape(NK, 128, NB2, 1024).transpose(1, 2, 0, 3)
        ).reshape(128, NB2 * NK * 1024)
        encB8 = encB.astype(f8)
        encP16 = np.ascontiguousarray(ec.transpose(1, 0, 2)).reshape(R, DE2).astype(
            np.float16
        )
        # exact f32 h-projection + bias, tiled per-partition: [128, (j, b)]
        h_proj = hidden[BL * c : BL * (c + 1), :] @ W_h.T + b  # [BL, DD]
        hb = _tile_rows(np.ascontiguousarray(h_proj.T), NO)  # [128, NO*BL]
        in_maps.append(
            {
                "encB8": encB8,
                "w8": w8_arr,
                "w16": w16_arr,
                "hb_in": np.ascontiguousarray(hb, dtype=np.float32),
                "v_pb": v_pb,
                "encP16": encP16,
                "ident16": ident,
                "iotac_in": iotac,
                "posb_in": posb,
            }
        )
    return in_maps


def kernel(hidden, encoder_outputs, W, b, v):
    """Full inputs in, full output out; 8-way batch-parallel inside."""
    from concourse.bass_utils import run_bass_kernel_spmd

    hidden = np.asarray(hidden, dtype=np.float32)
    enc = np.asarray(encoder_outputs, dtype=np.float32)
    W = np.asarray(W, dtype=np.float32)
    b = np.asarray(b, dtype=np.float32)
    v = np.asarray(v, dtype=np.float32)

    in_maps = _make_in_maps(hidden, enc, W, b, v)
    nc = _get_nc()
    res = run_bass_kernel_spmd(nc, in_maps, core_ids=list(range(NCORES)))
    out = np.concatenate([res.results[c]["probs"] for c in range(NCORES)], axis=0)
    return out.astype(np.float32)


# revision 32
# speedup vs baseline: 1.2876x; 1.2876x over previous
"""Bahdanau-attention scores kernel for Trainium2, 8-core data-parallel.

Computes softmax_s( v . tanh(W_h @ h[b] + W_e @ enc[s,b] + bias) ) for
B=32, S=2048, Dd=512, De2=1024, sharded 4 batches per NeuronCore.

Two-precision scheme (single launch):
  Pass 1 (fp8):  E^T = W_e8 @ enc8 on the PE in e4m3 DoubleRow mode
                 (K=256 per pass, 2x fp16 rate). tanh on ACT with
                 scale=1/128 dequant + per-partition h-projection bias,
                 v-weighted sum via DVE tree, scores via ones-matmul.
  Select:        per batch row, scores land as [16,128] (DRAM-roundtrip
                 relayout); top-8 of each 128-chunk via max_with_indices
                 = 128 candidates/row. fp8 score error (~0.2) only
                 matters for positions near the row max; top-8/chunk
                 covers everything with softmax weight > ~e^-6.
  Refine (fp16): gather the 128 selected enc rows (indirect DMA), PE
                 transpose, recompute scores in fp16, exp, and merge
                 back into the fp8 exp row via 8 predicated copies.
  Finalize:      row sum of merged exp -> reciprocal -> scale -> DMA.

The h-projection (hidden @ W_h^T + bias) is precomputed on host in
exact f32 and shipped as a per-partition bias table.
"""

import numpy as np

B = 32
S = 2048
DD = 512
DE2 = 1024
NCORES = 8
BL = B // NCORES  # 4 batches per core
R = BL * S  # 8192 rows per core
NK = DE2 // 128  # 8 k-chunks
NO = DD // 128  # 4 o-chunks
NB2 = R // 1024  # 8 DMA blocks of 1024 rows
EXP_OFF = -26.0  # softmax shift; scores observed in [-32, 27]
W8SCALE = 128.0  # fp8 weight pre-scale (keeps W_e out of e4m3 subnormals)
NWARM = 88

_CACHE = {}


def _build_bass():
    import concourse.bacc as bacc
    import concourse.mybir as mybir
    import concourse.tile as tile
    import concourse.bass as bass
    import concourse.bass_isa as bass_isa
    from concourse._compat import get_trn_type

    f32 = mybir.dt.float32
    f16 = mybir.dt.float16
    f8 = mybir.dt.float8e4
    i32 = mybir.dt.int32
    u32 = mybir.dt.uint32
    AF = mybir.ActivationFunctionType
    DR = mybir.MatmulPerfMode.DoubleRow

    nc = bacc.Bacc(get_trn_type() or "TRN2", target_bir_lowering=False, debug=False)

    encB8 = nc.dram_tensor("encB8", [128, NB2 * NK * 1024], f8, kind="ExternalInput")
    w8 = nc.dram_tensor("w8", [128, NO * NK * 128], f8, kind="ExternalInput")
    w16 = nc.dram_tensor("w16", [128, NO * NK * 128], f16, kind="ExternalInput")
    hb_in = nc.dram_tensor("hb_in", [128, NO * BL], f32, kind="ExternalInput")
    v_pb = nc.dram_tensor("v_pb", [128, NO], f32, kind="ExternalInput")
    encP16 = nc.dram_tensor("encP16", [R, DE2], f16, kind="ExternalInput")
    ident16 = nc.dram_tensor("ident16", [128, 128], f16, kind="ExternalInput")
    iotac_in = nc.dram_tensor("iotac_in", [16, 128], f32, kind="ExternalInput")
    posb_in = nc.dram_tensor("posb_in", [16, BL], f32, kind="ExternalInput")
    probs = nc.dram_tensor("probs", [BL, S], f32, kind="ExternalOutput")
    scr_dram = nc.dram_tensor("scr_dram", [BL, S], f32, kind="Internal")
    scl_dram = nc.dram_tensor("scl_dram", [BL, 128], f32, kind="Internal")
    idx_dram = nc.dram_tensor("idx_dram", [BL, 128], i32, kind="Internal")

    with tile.TileContext(nc) as tc:
        with (
            tc.tile_pool(name="const", bufs=1) as const,
            tc.tile_pool(name="encp", bufs=3) as encp,
            tc.tile_pool(name="etp", bufs=8) as etp,
            tc.tile_pool(name="prp", bufs=10) as prp,
            tc.tile_pool(name="refp", bufs=2) as refp,
            tc.tile_pool(name="pep", bufs=4, space="PSUM") as pep,
            tc.tile_pool(name="pmisc", bufs=2, space="PSUM") as pmisc,
            tc.tile_pool(name="pref", bufs=2, space="PSUM") as pref,
        ):
            # ---- PE warm-up: dummy matmuls while DMAs stream in ----
            warm_sb = const.tile([128, 128], f16, name="warm_sb")
            nc.any.memset(warm_sb[:], 0.0)
            wu_ps = pmisc.tile([128, 128], f32, name="wu_ps", tag="mi")
            for i in range(NWARM):
                nc.tensor.matmul(
                    wu_ps[:], warm_sb[:], warm_sb[:], start=True, stop=True
                )

            # ---- constants / weights (scalar queue) ----
            encB_v = encB8[:].rearrange("p (t k r) -> p t k r", t=NB2, k=NK)
            w8_sb = const.tile([128, NO, NK, 128], f8, name="w8_sb")
            w8_v = w8[:].rearrange("p (j k oo) -> p j k oo", j=NO, k=NK)
            for j in range(NO):
                nc.scalar.dma_start(w8_sb[:, j], w8_v[:, j])
            hb_sb = const.tile([128, NO, BL], f32, name="hb_sb")
            nc.scalar.dma_start(hb_sb[:], hb_in[:].rearrange("p (j b) -> p j b", j=NO))
            v_sb = const.tile([128, NO], f32, name="v_sb")
            nc.scalar.dma_start(v_sb[:], v_pb[:])
            w16_sb = const.tile([128, NO, NK, 128], f16, name="w16_sb")
            nc.scalar.dma_start(
                w16_sb[:], w16[:].rearrange("p (j k oo) -> p j k oo", j=NO, k=NK)
            )
            id_sb = const.tile([128, 128], f16, name="id_sb")
            nc.scalar.dma_start(id_sb[:], ident16[:])
            iotac = const.tile([16, 128], f32, name="iotac")
            nc.scalar.dma_start(iotac[:], iotac_in[:])
            posb = const.tile([16, BL], f32, name="posb")
            nc.scalar.dma_start(posb[:], posb_in[:])

            ones_v = const.tile([128, 1], f16, name="ones_v")
            nc.any.memset(ones_v[:], 1.0)
            ones16 = const.tile([16, 1], f32, name="ones16")
            nc.any.memset(ones16[:], 1.0)
            onesb = const.tile([1, 16], f32, name="onesb")
            nc.any.memset(onesb[:], 1.0)
            expoff16 = const.tile([16, 1], f32, name="expoff16")
            nc.any.memset(expoff16[:], EXP_OFF)
            scrow = [
                const.tile([1, S], f32, name=f"scrow{b}") for b in range(2)
            ]  # double-buffered per-row score rows

            def emit_score(st):
                p0, b0_, t0_ = st
                sc = pmisc.tile([1, 512], f32, name="sc", tag="mi")
                nc.tensor.matmul(sc[:], ones_v[:], p0[:], start=True, stop=True)
                nc.scalar.copy(scrow[b0_ % 2][0:1, 512 * t0_ : 512 * (t0_ + 1)], sc[:])

            def emit_refine_a1(b):
                row = scrow[b % 2]
                # relayout scores [1,2048] -> [16,128] via DRAM roundtrip
                nc.gpsimd.dma_start(scr_dram[b : b + 1], row[:])
                sc16 = refp.tile([16, 128], f32, name="sc16", tag="sc16")
                nc.gpsimd.dma_start(
                    sc16[:], scr_dram[b].rearrange("(p t) -> p t", p=16)
                )
                return sc16

            def emit_refine_a2(b, sc16):
                expo16 = refp.tile([16, 128], f32, name="expo16", tag="ex16")
                nc.scalar.activation(expo16[:], sc16[:], AF.Exp, bias=expoff16[:])
                # top-8 per 128-chunk
                m8 = refp.tile([16, 8], f32, name="m8", tag="m8")
                mi = refp.tile([16, 8], u32, name="mi", tag="mi8")
                nc.vector.max_with_indices(m8[:], mi[:], sc16[:])
                mif = refp.tile([16, 8], f32, name="mif", tag="mif")
                nc.vector.tensor_copy(mif[:], mi[:])
                idxgf = refp.tile([16, 8], f32, name="idxgf", tag="idxgf")
                nc.vector.tensor_scalar(
                    idxgf[:], mif[:], posb[:, b : b + 1], None, mybir.AluOpType.add
                )
                idxg = refp.tile([16, 8], i32, name="idxg", tag="idxg")
                nc.vector.tensor_copy(idxg[:], idxgf[:])
                # indices -> [128,1] via DRAM roundtrip, then gather enc rows
                nc.gpsimd.dma_start(
                    idx_dram[b].rearrange("(p j) -> p j", p=16), idxg[:]
                )
                idx128 = refp.tile([128, 1], i32, name="idx128", tag="i128")
                nc.gpsimd.dma_start(
                    idx128[:], idx_dram[b].rearrange("(p j) -> p j", p=128)
                )
                gath = refp.tile([128, DE2], f16, name="gath", tag="gath")
                nc.gpsimd.indirect_dma_start(
                    out=gath[:],
                    out_offset=None,
                    in_=encP16[:],
                    in_offset=bass.IndirectOffsetOnAxis(ap=idx128[:, :1], axis=0),
                )
                return (gath, expo16, mif)

            def emit_refine_b1(b, st):
                gath, expo16, mif = st
                # PE transpose to [128 feat, 128 pos] per k-chunk
                encsel = refp.tile([128, NK, 128], f16, name="encsel", tag="esel")
                for k in range(NK):
                    tp = pref.tile([128, 128], f16, name="tp", tag="rf")
                    nc.tensor.transpose(
                        tp[:], gath[:, 128 * k : 128 * (k + 1)], id_sb[:]
                    )
                    nc.vector.tensor_copy(encsel[:, k], tp[:])
                # fp16 recompute of the 128 selected scores
                ret = []
                for j in range(NO):
                    rpe = pref.tile([128, 128], f32, name="rpe", tag="rf")
                    for k in range(NK):
                        nc.tensor.matmul(
                            rpe[:],
                            w16_sb[:, j, k, :],
                            encsel[:, k, :],
                            start=(k == 0),
                            stop=(k == NK - 1),
                        )
                    rt = refp.tile([128, 128], f16, name="rt", tag=f"rt{j}")
                    nc.scalar.activation(
                        rt[:], rpe[:], AF.Tanh, bias=hb_sb[:, j, b : b + 1]
                    )
                    ret.append(rt)
                scsel = pref.tile([1, 128], f32, name="scsel", tag="rf")
                for j in range(NO):
                    nc.tensor.matmul(
                        scsel[:],
                        v16_sb[:, j : j + 1],
                        ret[j][:],
                        start=(j == 0),
                        stop=(j == NO - 1),
                    )
                scselS = refp.tile([1, 128], f32, name="scselS", tag="sclS")
                nc.scalar.copy(scselS[:], scsel[:])
                # [1,128] -> [16,8] roundtrip, exp, merge into expo16
                nc.gpsimd.dma_start(scl_dram[b : b + 1], scselS[:])
                scs16 = refp.tile([16, 8], f32, name="scs16", tag="scs16")
                nc.gpsimd.dma_start(
                    scs16[:], scl_dram[b].rearrange("(p j) -> p j", p=16)
                )
                return (scs16, expo16, mif)

            def emit_refine_b2(b, st):
                scs16, expo16, mif = st
                es16 = refp.tile([16, 8], f32, name="es16", tag="es16")
                nc.scalar.activation(es16[:], scs16[:], AF.Exp, bias=expoff16[:])
                for j in range(8):
                    mj = refp.tile([16, 128], mybir.dt.int32, name="mj", tag="mj")
                    nc.vector.tensor_scalar(
                        mj[:], iotac[:], mif[:, j : j + 1], None,
                        mybir.AluOpType.is_equal,
                    )
                    nc.vector.copy_predicated(
                        expo16[:], mj[:], es16[:, j : j + 1].to_broadcast([16, 128])
                    )
                return expo16

            def emit_refine_b3(b, expo16):
                # row sum -> reciprocal -> broadcast -> normalize -> out
                rsum = refp.tile([16, 1], f32, name="rsum", tag="rsum")
                nc.vector.reduce_sum(rsum[:], expo16[:], axis=mybir.AxisListType.X)
                tot = pref.tile([1, 1], f32, name="tot", tag="rf")
                nc.tensor.matmul(tot[:], ones16[:], rsum[:], start=True, stop=True)
                totS = refp.tile([1, 1], f32, name="totS", tag="totS")
                nc.vector.tensor_copy(totS[:], tot[:])
                rec = refp.tile([1, 1], f32, name="rec", tag="rec")
                nc.vector.reciprocal(rec[:], totS[:])
                recb = pref.tile([16, 1], f32, name="recb", tag="rf")
                nc.tensor.matmul(recb[:], onesb[:], rec[:], start=True, stop=True)
                recbS = refp.tile([16, 1], f32, name="recbS", tag="rcbS")
                nc.vector.tensor_copy(recbS[:], recb[:])
                probs16 = refp.tile([16, 128], f32, name="probs16", tag="p16")
                nc.vector.tensor_scalar_mul(probs16[:], expo16[:], recbS[:])
                nc.scalar.dma_start(
                    probs[b].rearrange("(p t) -> p t", p=16), probs16[:]
                )

            v16_sb = const.tile([128, NO], f16, name="v16_sb")
            nc.vector.tensor_copy(v16_sb[:], v_sb[:])

            # ---- main loop: 8 DMA blocks of 1024 rows (= half a batch).
            # All small/dependent ops are emitted AFTER each block's main
            # matmuls so in-order engines never stall ahead of bulk work.
            # refine(r) phases: A1@end(2r+2) A2@end(2r+3) B1@end(2r+4)
            # B2@end(2r+5); later phases spill into the tail.
            pending = []
            ref_st = {}
            for t2 in range(NB2):
                enc_t = encp.tile([128, NK, 1024], f8, name="enc_t", tag="enc")
                nc.sync.dma_start(enc_t[:], encB_v[:, t2])
                b = t2 // 2
                prodacc = [None, None]
                for j in range(NO):
                    pe_h = [
                        pep.tile([128, 512], f32, name="pe", tag="pe")
                        for _ in range(2)
                    ]
                    for kk in range(NK // 2):
                        for h in range(2):
                            nc.tensor.matmul(
                                pe_h[h][:],
                                w8_sb[:, j, 2 * kk : 2 * kk + 2, :],
                                enc_t[:, 2 * kk : 2 * kk + 2, 512 * h : 512 * (h + 1)],
                                start=(kk == 0),
                                stop=(kk == NK // 2 - 1),
                                perf_mode=DR,
                            )
                    for h in range(2):
                        et = etp.tile([128, 512], f16, name="et", tag="et")
                        nc.scalar.activation(
                            et[:],
                            pe_h[h][:],
                            AF.Tanh,
                            bias=hb_sb[:, j, b : b + 1],
                            scale=1.0 / W8SCALE,
                        )
                        if j == 0:
                            pa = prp.tile([128, 512], f16, name="pa", tag="pa")
                            nc.vector.tensor_scalar_mul(pa[:], et[:], v_sb[:, 0:1])
                            prodacc[h] = pa
                        else:
                            pj = prp.tile([128, 512], f16, name="pj", tag="pj")
                            nc.vector.tensor_scalar_mul(
                                pj[:], et[:], v_sb[:, j : j + 1]
                            )
                            nc.vector.tensor_add(
                                prodacc[h][:], prodacc[h][:], pj[:]
                            )
                # ---- block-end emissions (deps are a full block old) ----
                for st in pending:
                    emit_score(st)
                pending = []
                for h in range(2):
                    t_i = (t2 % 2) * 2 + h
                    pending.append((prodacc[h], b, t_i))
                r_a1 = (t2 - 2) // 2 if t2 >= 2 and t2 % 2 == 0 else None
                r_a2 = (t2 - 3) // 2 if t2 >= 3 and t2 % 2 == 1 else None
                r_b1 = (t2 - 4) // 2 if t2 >= 4 and t2 % 2 == 0 else None
                r_b2 = (t2 - 5) // 2 if t2 >= 5 and t2 % 2 == 1 else None
                if r_a1 is not None:
                    ref_st[("a", r_a1)] = emit_refine_a1(r_a1)
                if r_b1 is not None:
                    ref_st[("b", r_b1)] = emit_refine_b1(
                        r_b1, ref_st.pop(("a2", r_b1))
                    )
                if r_a2 is not None:
                    ref_st[("a2", r_a2)] = emit_refine_a2(
                        r_a2, ref_st.pop(("a", r_a2))
                    )
                if r_b2 is not None:
                    ref_st[("e", r_b2)] = emit_refine_b2(
                        r_b2, ref_st.pop(("b", r_b2))
                    )
                r_b3 = (t2 - 6) // 2 if t2 >= 6 and t2 % 2 == 0 else None
                if r_b3 is not None:
                    emit_refine_b3(r_b3, ref_st.pop(("e", r_b3)))

            # ---- tail: scores of block 7, then refine rows 2 and 3 ----
            for st in pending:
                emit_score(st)
            emit_refine_b3(1, ref_st.pop(("e", 1)))
            ref_st[("b", 2)] = emit_refine_b1(2, ref_st.pop(("a2", 2)))
            sc16_3 = emit_refine_a1(BL - 1)
            st3 = emit_refine_a2(BL - 1, sc16_3)
            e2 = emit_refine_b2(2, ref_st.pop(("b", 2)))
            e3 = emit_refine_b2(BL - 1, emit_refine_b1(BL - 1, st3))
            emit_refine_b3(2, e2)
            emit_refine_b3(BL - 1, e3)

    nc.compile()
    return nc


def _get_nc():
    if "nc" not in _CACHE:
        _CACHE["nc"] = _build_bass()
    return _CACHE["nc"]


def _tile_rows(mat_t, nchunk):
    # [nchunk*128, F] -> [128, nchunk*F] with out[p, c*F+f] = mat_t[128c+p, f]
    n, F = mat_t.shape
    assert n == nchunk * 128
    return np.ascontiguousarray(
        mat_t.reshape(nchunk, 128, F).transpose(1, 0, 2)
    ).reshape(128, nchunk * F)


def _make_in_maps(hidden, enc, W, b, v):
    import ml_dtypes

    f8 = ml_dtypes.float8_e4m3
    W_h = W[:, :DD]
    W_e = W[:, DD:]
    # w[p, j, k, oo] = W_e[128j+oo, 128k+p]
    w_lay = np.ascontiguousarray(
        W_e.reshape(NO, 128, NK, 128).transpose(3, 0, 2, 1)
    ).reshape(128, NO * NK * 128)
    w8_arr = (w_lay * W8SCALE).astype(f8)
    w16_arr = w_lay.astype(np.float16)
    v_pb = np.ascontiguousarray(v.reshape(NO, 128).T).astype(np.float32)
    ident = np.eye(128, dtype=np.float16)
    iotac = np.broadcast_to(
        np.arange(128, dtype=np.float32), (16, 128)
    ).copy()
    posb = (
        2048.0 * np.arange(BL)[None, :] + 128.0 * np.arange(16)[:, None]
    ).astype(np.float32)
    in_maps = []
    for c in range(NCORES):
        ec = enc[:, BL * c : BL * (c + 1), :]  # [S, BL, DE2]
        encT = np.ascontiguousarray(ec.transpose(2, 1, 0)).reshape(DE2, R)
        encB = np.ascontiguousarray(
            encT.reshape(NK, 128, NB2, 1024).transpose(1, 2, 0, 3)
        ).reshape(128, NB2 * NK * 1024)
        encB8 = encB.astype(f8)
        encP16 = np.ascontiguousarray(ec.transpose(1, 0, 2)).reshape(R, DE2).astype(
            np.float16
        )
        # exact f32 h-projection + bias, tiled per-partition: [128, (j, b)]
        h_proj = hidden[BL * c : BL * (c + 1), :] @ W_h.T + b  # [BL, DD]
        hb = _tile_rows(np.ascontiguousarray(h_proj.T), NO)  # [128, NO*BL]
        in_maps.append(
            {
                "encB8": encB8,
                "w8": w8_arr,
                "w16": w16_arr,
                "hb_in": np.ascontiguousarray(hb, dtype=np.float32),
                "v_pb": v_pb,
                "encP16": encP16,
                "ident16": ident,
                "iotac_in": iotac,
                "posb_in": posb,
            }
        )
    return in_maps


def kernel(hidden, encoder_outputs, W, b, v):
    """Full inputs in, full output out; 8-way batch-parallel inside."""
    from concourse.bass_utils import run_bass_kernel_spmd

    hidden = np.asarray(hidden, dtype=np.float32)
    enc = np.asarray(encoder_outputs, dtype=np.float32)
    W = np.asarray(W, dtype=np.float32)
    b = np.asarray(b, dtype=np.float32)
    v = np.asarray(v, dtype=np.float32)

    in_maps = _make_in_maps(hidden, enc, W, b, v)
    nc = _get_nc()
    res = run_bass_kernel_spmd(nc, in_maps, core_ids=list(range(NCORES)))
    out = np.concatenate([res.results[c]["probs"] for c in range(NCORES)], axis=0)
    return out.astype(np.float32)
